# revision 4
# baseline (speedup 1.0000x reference)
"""Trainium2 Bass kernel for nn_ImprintedModel (retrieval_knn).

Computes y[c, b] = max over the 32 proxies p of class c of
    (w1[p] / ||w1[p]||) . (data[b] / ||data[b]||)
for data [4096, 512], w1 [64000, 512] (2000 classes x 32 proxies),
output [2000, 4096] fp32.

Sharding: w1 rows (and hence classes) split across 8 cores (8000 rows =
250 classes per core); data replicated. Each core computes its 250
output rows for all 4096 batch columns; host concatenates/transposes.

Strategy (v3, fp8 DoubleRow + split PSUM drain):
  * Host prep: l2-normalize data and w rows, scale by S=16, quantize to
    fp8e4m3, and lay out both operands pre-transposed as
    [half, 128, 2, cols] so the contraction (E=512) maps onto
    2 halves x (2 k-subtiles x 128 partitions). Measured end-to-end
    rel err of the fp8 path on the real inputs: ~1.5e-2 (gate 2e-2).
  * PE: fp8 DoubleRow matmuls only (0.5 cyc/row, K=256 per instr) --
    2 accumulating matmuls per (batch m-tile, 512-row w tile) into
    4-bank PSUM groups of 64 classes.
  * PSUM drain (the floor: only DVE+ACT have PSUM ports, 0.96/1.2 GHz):
    - 89 "direct" classes/m-tile: DVE tensor_reduce straight from PSUM.
    - 161 "scan" classes/m-tile: ACT copies PSUM->SBUF as fp16 with a
      +SHIFT bias, then one DVE masked max-scan per m-tile
      (state = (mask*state) max x, mask=0 at class starts) runs at the
      4x DVE rate; class maxes sit at the last column of each group and
      are pulled out by a strided output DMA.
  * Class order is permuted device-side; the host un-permutes, unshifts
    and applies the 1/S^2 descale when assembling the full output.
"""

import numpy as np

# Problem shapes (hardcoded; harness always calls with these).
B = 4096
E = 512
C = 2000
PROXIES = 32
P = C * PROXIES
N_CORES = 8
P_SHARD = P // N_CORES      # 8000 w rows per core
C_SHARD = C // N_CORES      # 250 classes per core
EPS = 1e-12

PE_TILE = 128
NW = 512                    # w rows per matmul (one psum bank)
SCALE = 16.0                # fp8 pre-scale; output descaled by 1/S^2
SHIFT = 128.0               # scan-route positivity shift (host-removed)

# PSUM groups per m-tile: (row offset, rows). 4 banks each.
GROUPS = [(0, 2048), (2048, 2048), (4096, 2048), (6144, 1856)]
NCLS = [64, 64, 64, 58]
# Per-group class split: first CD -> DVE tensor_reduce, rest -> scan.
CD = [23, 22, 22, 22]
CA = [n - d for n, d in zip(NCLS, CD)]
ND = sum(CD)                # 89 direct classes
NA = sum(CA)                # 161 scan classes
DCOL = np.cumsum([0] + CD).tolist()
ACOL = np.cumsum([0] + CA).tolist()


def build_bass_kernel():
    from concourse import bacc, mybir
    from concourse.tile import TileContext

    f32 = mybir.dt.float32
    f16 = mybir.dt.float16
    f8 = mybir.dt.float8e4
    AX = mybir.AxisListType
    OP = mybir.AluOpType
    AF = mybir.ActivationFunctionType
    PM = mybir.MatmulPerfMode.DoubleRow

    MT = B // PE_TILE               # 32 batch m-tiles

    nc = bacc.Bacc("TRN2", target_bir_lowering=False, debug=False)
    w8_d = nc.dram_tensor("w8", [2, PE_TILE, 2, P_SHARD], f8,
                          kind="ExternalInput")
    d8_d = nc.dram_tensor("d8", [2, PE_TILE, 2, B], f8,
                          kind="ExternalInput")
    mask_d = nc.dram_tensor("mask", [PE_TILE, NA * PROXIES], f16,
                            kind="ExternalInput")
    outa_d = nc.dram_tensor("outa", [B, ND], f32, kind="ExternalOutput")
    outb_d = nc.dram_tensor("outb", [B, NA], f16, kind="ExternalOutput")

    with TileContext(nc) as tc:
        with tc.tile_pool(name="sbuf", bufs=1) as sb, \
             tc.tile_pool(name="mmps", bufs=2, space="PSUM") as psm:

            W8 = [sb.tile([PE_TILE, 2, P_SHARD], f8, tag=f"w8_{h}",
                          name=f"w8_{h}") for h in range(2)]
            D8 = [sb.tile([PE_TILE, 2, B], f8, tag=f"d8_{h}",
                          name=f"d8_{h}") for h in range(2)]
            mask = sb.tile([PE_TILE, NA * PROXIES], f16, tag="mask",
                           name="mask")
            # data + mask first (needed from the first m-tile), w by
            # group so m=0 can start before the whole shard lands
            for h in range(2):
                nc.sync.dma_start(D8[h][:], d8_d[h])
            nc.sync.dma_start(mask[:], mask_d[:])
            for g0, gw in GROUPS:
                nc.sync.dma_start(W8[0][:, :, g0:g0 + gw],
                                  w8_d[0, :, :, g0:g0 + gw])
                nc.scalar.dma_start(W8[1][:, :, g0:g0 + gw],
                                    w8_d[1, :, :, g0:g0 + gw])

            out_sb = [sb.tile([PE_TILE, ND], f32, tag=f"osb{m}",
                              name=f"osb{m}") for m in range(MT)]

            for m in range(MT):
                tb = sb.tile([PE_TILE, NA * PROXIES], f16, tag="tb",
                             bufs=2, name="tb")
                tb3 = tb.rearrange("p (c x) -> p c x", x=PROXIES)
                for g, (g0, gw) in enumerate(GROUPS):
                    ps = psm.tile([PE_TILE, 2048], f32, tag="ps", name="ps")
                    off = 0
                    while off < gw:
                        nw = min(NW, gw - off)
                        for h in range(2):
                            nc.tensor.matmul(
                                ps[:, off:off + nw],
                                D8[h][:, :, m * 128:(m + 1) * 128],
                                W8[h][:, :, g0 + off:g0 + off + nw],
                                start=(h == 0),
                                stop=(h == 1),
                                perf_mode=PM,
                            )
                        off += nw
                    cd, ca = CD[g], CA[g]
                    nc.vector.tensor_reduce(
                        out_sb[m][:, DCOL[g]:DCOL[g] + cd],
                        ps[:, :cd * 32].rearrange("p (c x) -> p c x", x=32),
                        axis=AX.X,
                        op=OP.max,
                    )
                    nc.scalar.activation(
                        tb3[:, ACOL[g]:ACOL[g] + ca, :],
                        ps[:, cd * 32:gw].rearrange("p (c x) -> p c x", x=32),
                        AF.Copy, bias=SHIFT, scale=1.0,
                    )
                # one masked max-scan per m-tile folds each 32-wide
                # class group to its running max (result in column 31)
                sc = sb.tile([PE_TILE, NA * PROXIES], f16, tag="sc",
                             bufs=2, name="sc")
                nc.vector.tensor_tensor_scan(sc[:], mask[:], tb[:], 0.0,
                                             op0=OP.mult, op1=OP.max)
                nc.sync.dma_start(outa_d[m * 128:(m + 1) * 128, :],
                                  out_sb[m][:])
                nc.sync.dma_start(
                    outb_d[m * 128:(m + 1) * 128, :],
                    sc[:].rearrange("p (c x) -> p c x",
                                    x=PROXIES)[:, :, PROXIES - 1])

    nc.compile()
    return nc


_NC_CACHE = {}


def _get_nc(key):
    if key not in _NC_CACHE:
        _NC_CACHE[key] = build_bass_kernel()
    return _NC_CACHE[key]


def _route():
    """Per true class (0..249): (is_direct, device column)."""
    routes = []
    for c in range(C_SHARD):
        g = min(c // 64, 3)
        j = c - 64 * g
        if j < CD[g]:
            routes.append((True, DCOL[g] + j))
        else:
            routes.append((False, ACOL[g] + (j - CD[g])))
    return routes


def _fp8_pack(x):
    """[rows, 512] fp32 -> [2, 128, 2, rows] fp8e4m3 (pre-transposed)."""
    import ml_dtypes
    x8 = (x * SCALE).astype(ml_dtypes.float8_e4m3fn)
    # [rows, e] -> [e, rows] -> [h, i, k, rows] -> [h, k, i, rows]
    return np.ascontiguousarray(
        x8.T.reshape(2, 2, 128, x.shape[0]).transpose(0, 2, 1, 3))


def kernel(data, w1, segment_ids=None):
    """Full-input entry point: shards internally across 8 NeuronCores."""
    from concourse.bass_utils import run_bass_kernel_spmd

    data = np.ascontiguousarray(np.asarray(data), dtype=np.float32)
    w1 = np.ascontiguousarray(np.asarray(w1), dtype=np.float32)
    assert data.shape == (B, E) and w1.shape == (P, E)

    dn = data / np.maximum(
        np.linalg.norm(data, axis=1, keepdims=True), EPS)
    wn = w1 / np.maximum(np.linalg.norm(w1, axis=1, keepdims=True), EPS)
    d8 = _fp8_pack(dn)
    mask = np.ones((PE_TILE, NA * PROXIES), dtype=np.float16)
    mask[:, ::PROXIES] = 0.0

    nc = _get_nc("full")
    in_maps = [
        {"d8": d8, "mask": mask,
         "w8": _fp8_pack(wn[i * P_SHARD:(i + 1) * P_SHARD])}
        for i in range(N_CORES)
    ]
    res = run_bass_kernel_spmd(nc, in_maps, core_ids=list(range(N_CORES)))

    inv_s2 = np.float32(1.0 / (SCALE * SCALE))
    routes = _route()
    a_rows = np.array([col for d, col in routes if d])
    b_rows = np.array([col for d, col in routes if not d])
    a_idx = np.array([c for c, (d, _) in enumerate(routes) if d])
    b_idx = np.array([c for c, (d, _) in enumerate(routes) if not d])
    out = np.empty((C, B), dtype=np.float32)
    for i in range(N_CORES):
        blk = out[i * C_SHARD:(i + 1) * C_SHARD]
        ra = res.results[i]["outa"].T         # [ND, B] fp32 (scaled)
        rb = res.results[i]["outb"].T         # [NA, B] fp16 (scaled+shift)
        blk[a_idx] = ra[a_rows] * inv_s2
        blk[b_idx] = (rb[b_rows].astype(np.float32) - SHIFT) * inv_s2
    return out


# revision 5
# speedup vs baseline: 1.8336x; 1.8336x over previous
"""Trainium2 Bass kernel for nn_ImprintedModel (retrieval_knn).

Computes y[c, b] = max over the 32 proxies p of class c of
    (w1[p] / ||w1[p]||) . (data[b] / ||data[b]||)
for data [4096, 512], w1 [64000, 512] (2000 classes x 32 proxies),
output [2000, 4096] fp32.

Sharding: w1 rows (and hence classes) split across 8 cores (8000 rows =
250 classes per core); data replicated. Each core computes its 250
output rows for all 4096 batch columns; host concatenates/transposes.

Strategy (v4, fp8 DoubleRow + split PSUM drain):
  * Host prep: l2-normalize data and w rows, scale by S=16, quantize to
    fp8e4m3, and lay out both operands pre-transposed as
    [half, 128, 2, cols] so the contraction (E=512) maps onto
    2 halves x (2 k-subtiles x 128 partitions). Measured end-to-end
    rel err of the fp8 path on the real inputs: ~1.5e-2 (gate 2e-2).
  * PE: fp8 DoubleRow matmuls only (0.5 cyc/row, K=256 per instr) --
    2 accumulating matmuls per (batch m-tile, 512-row w tile) into
    4-bank PSUM groups of 64 classes (4 groups per m-tile).
  * PSUM drain (the floor: only DVE+ACT have PSUM ports, 0.96/1.2 GHz;
    instruction-count overheads matter, so each group is drained by ONE
    engine in ONE instruction):
    - group 0 (classes 0..63): DVE tensor_reduce straight from PSUM
      into out_sb fp32.
    - groups 1-3 (classes 64..249): ACT copies PSUM->SBUF fp16 (no 2x
      modes exist for reduce/copy from PSUM; ACT is the faster copier),
      then a 5-level DVE tensor_tensor max ladder (fp16 pairs run at
      the 2x_1p DVE rate) folds 32 -> 1, writing fp32 into out_sb.
  * One contiguous [128, 250] fp32 output DMA per m-tile; natural class
    order throughout (no permutation); host just transposes + descales.
"""

import numpy as np

# Problem shapes (hardcoded; harness always calls with these).
B = 4096
E = 512
C = 2000
PROXIES = 32
P = C * PROXIES
N_CORES = 8
P_SHARD = P // N_CORES      # 8000 w rows per core
C_SHARD = C // N_CORES      # 250 classes per core
EPS = 1e-12

PE_TILE = 128
NW = 512                    # w rows per matmul (one psum bank)
SCALE = 16.0                # fp8 pre-scale; output descaled by 1/S^2

# PSUM groups per m-tile: (row offset, rows, n classes). 4 banks each.
GROUPS = [(0, 2048, 64), (2048, 2048, 64), (4096, 2048, 64),
          (6144, 1856, 58)]
ND = 64                     # group-0 classes: DVE direct reduce
NA = C_SHARD - ND           # 186 ladder classes (groups 1-3)


def build_bass_kernel():
    from concourse import bacc, mybir
    from concourse.tile import TileContext

    f32 = mybir.dt.float32
    f16 = mybir.dt.float16
    f8 = mybir.dt.float8e4
    AX = mybir.AxisListType
    OP = mybir.AluOpType
    AF = mybir.ActivationFunctionType
    PM = mybir.MatmulPerfMode.DoubleRow

    MT = B // PE_TILE               # 32 batch m-tiles

    nc = bacc.Bacc("TRN2", target_bir_lowering=False, debug=False)
    w8_d = nc.dram_tensor("w8", [2, PE_TILE, 2, P_SHARD], f8,
                          kind="ExternalInput")
    d8_d = nc.dram_tensor("d8", [2, PE_TILE, 2, B], f8,
                          kind="ExternalInput")
    out_d = nc.dram_tensor("out", [B, C_SHARD], f32, kind="ExternalOutput")

    with TileContext(nc) as tc:
        with tc.tile_pool(name="sbuf", bufs=1) as sb, \
             tc.tile_pool(name="mmps", bufs=2, space="PSUM") as psm:

            W8 = [sb.tile([PE_TILE, 2, P_SHARD], f8, tag=f"w8_{h}",
                          name=f"w8_{h}") for h in range(2)]
            D8 = [sb.tile([PE_TILE, 2, B], f8, tag=f"d8_{h}",
                          name=f"d8_{h}") for h in range(2)]
            # data first (needed by every matmul), w by group so the
            # m=0 loop can start before the whole shard lands
            for h in range(2):
                nc.sync.dma_start(D8[h][:], d8_d[h])
            for g0, gw, _ in GROUPS:
                nc.sync.dma_start(W8[0][:, :, g0:g0 + gw],
                                  w8_d[0, :, :, g0:g0 + gw])
                nc.scalar.dma_start(W8[1][:, :, g0:g0 + gw],
                                    w8_d[1, :, :, g0:g0 + gw])

            out_sb = [sb.tile([PE_TILE, C_SHARD], f32, tag=f"osb{m}",
                              name=f"osb{m}") for m in range(MT)]

            for m in range(MT):
                tb = sb.tile([PE_TILE, NA, PROXIES], f16, tag="tb",
                             bufs=2, name="tb")
                acol = 0
                for g, (g0, gw, ncls) in enumerate(GROUPS):
                    ps = psm.tile([PE_TILE, 2048], f32, tag="ps", name="ps")
                    off = 0
                    while off < gw:
                        nw = min(NW, gw - off)
                        for h in range(2):
                            nc.tensor.matmul(
                                ps[:, off:off + nw],
                                D8[h][:, :, m * 128:(m + 1) * 128],
                                W8[h][:, :, g0 + off:g0 + off + nw],
                                start=(h == 0),
                                stop=(h == 1),
                                perf_mode=PM,
                            )
                        off += nw
                    if g == 0:
                        nc.vector.tensor_reduce(
                            out_sb[m][:, 0:ND],
                            ps[:, :gw].rearrange("p (c x) -> p c x", x=32),
                            axis=AX.X,
                            op=OP.max,
                        )
                    else:
                        nc.scalar.copy(
                            tb[:, acol:acol + ncls, :],
                            ps[:, :gw].rearrange("p (c x) -> p c x", x=32),
                        )
                        acol += ncls
                # 5-level pairwise max ladder 32 -> 1 (fp16, 2x DVE rate)
                t16 = sb.tile([PE_TILE, NA, 16], f16, tag="t16", bufs=2,
                              name="t16")
                nc.vector.tensor_max(t16[:], tb[:, :, 0:16], tb[:, :, 16:32])
                t8 = sb.tile([PE_TILE, NA, 8], f16, tag="t8", bufs=2,
                             name="t8")
                nc.vector.tensor_max(t8[:], t16[:, :, 0:8], t16[:, :, 8:16])
                t4 = sb.tile([PE_TILE, NA, 4], f16, tag="t4", bufs=2,
                             name="t4")
                nc.vector.tensor_max(t4[:], t8[:, :, 0:4], t8[:, :, 4:8])
                t2 = sb.tile([PE_TILE, NA, 2], f16, tag="t2", bufs=2,
                             name="t2")
                nc.vector.tensor_max(t2[:], t4[:, :, 0:2], t4[:, :, 2:4])
                nc.vector.tensor_max(out_sb[m][:, ND:C_SHARD],
                                     t2[:, :, 0], t2[:, :, 1])
                nc.sync.dma_start(out_d[m * 128:(m + 1) * 128, :],
                                  out_sb[m][:])

    nc.compile()
    return nc


_NC_CACHE = {}


def _get_nc(key):
    if key not in _NC_CACHE:
        _NC_CACHE[key] = build_bass_kernel()
    return _NC_CACHE[key]


def _fp8_pack(x):
    """[rows, 512] fp32 -> [2, 128, 2, rows] fp8e4m3 (pre-transposed)."""
    import ml_dtypes
    x8 = (x * SCALE).astype(ml_dtypes.float8_e4m3fn)
    # [rows, e] -> [e, rows] -> [h, i, k, rows] -> [h, k, i, rows]
    return np.ascontiguousarray(
        x8.T.reshape(2, 2, 128, x.shape[0]).transpose(0, 2, 1, 3))


def kernel(data, w1, segment_ids=None):
    """Full-input entry point: shards internally across 8 NeuronCores."""
    from concourse.bass_utils import run_bass_kernel_spmd

    data = np.ascontiguousarray(np.asarray(data), dtype=np.float32)
    w1 = np.ascontiguousarray(np.asarray(w1), dtype=np.float32)
    assert data.shape == (B, E) and w1.shape == (P, E)

    dn = data / np.maximum(
        np.linalg.norm(data, axis=1, keepdims=True), EPS)
    wn = w1 / np.maximum(np.linalg.norm(w1, axis=1, keepdims=True), EPS)
    d8 = _fp8_pack(dn)

    nc = _get_nc("full")
    in_maps = [
        {"d8": d8, "w8": _fp8_pack(wn[i * P_SHARD:(i + 1) * P_SHARD])}
        for i in range(N_CORES)
    ]
    res = run_bass_kernel_spmd(nc, in_maps, core_ids=list(range(N_CORES)))

    inv_s2 = np.float32(1.0 / (SCALE * SCALE))
    out = np.empty((C, B), dtype=np.float32)
    for i in range(N_CORES):
        out[i * C_SHARD:(i + 1) * C_SHARD, :] = \
            res.results[i]["out"].T * inv_s2
    return out
